# revision 1
# baseline (speedup 1.0000x reference)
"""CrossAttentionFusion Trainium2 kernel (nn_CrossAttentionFusion__45561013076033).

Full inputs -> full output. Sharding: 8 cores, core c handles batch b=c//2,
query-half h=c%2 (2048 of 4096 queries). Each core holds the full [256,4096]
cnn feature map of its batch (keys/values), its query-half of the transformer
features, and replicated weights.

Per-core dataflow (channel-major [C, N] layouts throughout):
  Q = (Wq X_trf + bq)/16          [256, 2048]  fp32r matmuls
  K = Wk X_cnn + bk               [256, 4096]
  V^T = X_cnn^T Wv^T              [4096, 256]  (bv folded into conv bias)
  per 128-query block:
    S = Q_blk^T K                 [128, 4096]  fp32r
    P = exp(S) (no max-sub; scores are O(1)), row sums via ACT accum_out
    PT = P^T diag(256/rowsum)     via fp16 matmul with scaled identity
  per 256-query superblock:
    A' = V^T^T PT = 256 * attended_norm   [256, 256]  fp16 matmuls, fp32 acc
    out = Wf1 X_trf + (Wf2/256) A' + (bf + Wf2 bv)    fp32r
"""

import numpy as np

B, C, H, W = 4, 256, 64, 64
N = H * W            # 4096 tokens
NCORES = 8
QH = N // 2          # 2048 queries per core
CT = C // 128        # 2 channel tiles
KC = N // 512        # 8 key chunks of 512
NQB = QH // 128      # 16 query blocks per core
NSB = QH // 512      # 4 superblocks per core
NKT = N // 128       # 32 key tiles

_CACHE = {}


def _build(bench_reps=None, dma_outside=False):
    import contextlib
    import concourse.bass as bass
    import concourse.mybir as mybir
    import concourse.tile as tile
    from concourse import bacc
    from concourse.masks import make_identity

    f32 = mybir.dt.float32
    f32r = mybir.dt.float32r
    f16 = mybir.dt.float16
    AF = mybir.ActivationFunctionType

    nc = bacc.Bacc("TRN2", target_bir_lowering=False, debug=True)

    XQ = nc.dram_tensor("xq", [C, QH], f32, kind="ExternalInput")
    XC = nc.dram_tensor("xc", [C, N], f32, kind="ExternalInput")
    WQT = nc.dram_tensor("wqt", [C, C], f32, kind="ExternalInput")
    WKT = nc.dram_tensor("wkt", [C, C], f32, kind="ExternalInput")
    WVT = nc.dram_tensor("wvt", [C, C], f32, kind="ExternalInput")
    WFT = nc.dram_tensor("wft", [2 * C, C], f32, kind="ExternalInput")
    BQ = nc.dram_tensor("bq", [C], f32, kind="ExternalInput")
    BK = nc.dram_tensor("bk", [C], f32, kind="ExternalInput")
    BF = nc.dram_tensor("bf", [C], f32, kind="ExternalInput")
    OUT = nc.dram_tensor("out", [C, QH], f32, kind="ExternalOutput")

    xq_d = XQ.ap().bitcast(f32r).rearrange("(t p) n -> p t n", p=128)
    xc_d = XC.ap().bitcast(f32r).rearrange("(t p) n -> p t n", p=128)
    wq_d = WQT.ap().bitcast(f32r).rearrange("(t p) d -> p t d", p=128)
    wk_d = WKT.ap().bitcast(f32r).rearrange("(t p) d -> p t d", p=128)
    wv_d = WVT.ap().bitcast(f32r).rearrange("(t p) d -> p t d", p=128)
    wf_d = WFT.ap().bitcast(f32r).rearrange("(t p) d -> p t d", p=128)
    out_d = OUT.ap().rearrange("(t p) n -> p t n", p=128)

    with tile.TileContext(nc) as tc:
        with tc.tile_pool(name="persist", bufs=1) as per, \
             tc.tile_pool(name="soft", bufs=2) as soft, \
             tc.tile_pool(name="ptp", bufs=1) as ptp, \
             tc.tile_pool(name="outp", bufs=2) as outp, \
             tc.tile_pool(name="mm", bufs=2, space="PSUM") as mmp, \
             tc.tile_pool(name="tp", bufs=2, space="PSUM") as tpp, \
             tc.tile_pool(name="av", bufs=2, space="PSUM") as avp:

            # ---- persistent tiles ----
            xq_sb = per.tile([128, CT, QH], f32r)
            xc_sb = per.tile([128, CT, N], f32r)
            wq_sb = per.tile([128, CT, C], f32r)
            wk_sb = per.tile([128, CT, C], f32r)
            wv_sb = per.tile([128, CT, C], f32r)
            wf_sb = per.tile([128, 2 * CT, C], f32r)
            bq_sb = per.tile([128, CT], f32)
            bk_sb = per.tile([128, CT], f32)
            bf_sb = per.tile([128, CT], f32)
            q_sb = per.tile([128, CT, QH], f32r)
            k_sb = per.tile([128, CT, N], f32r)
            vt_sb = per.tile([128, NKT, C], f16)
            ident = per.tile([128, 128], f16)

            nc.sync.dma_start(wq_sb[:], wq_d)
            nc.sync.dma_start(wk_sb[:], wk_d)
            nc.sync.dma_start(wv_sb[:], wv_d)
            nc.sync.dma_start(wf_sb[:], wf_d)
            nc.sync.dma_start(bq_sb[:], BQ.ap().rearrange("(t p) -> p t", p=128))
            nc.sync.dma_start(bk_sb[:], BK.ap().rearrange("(t p) -> p t", p=128))
            nc.sync.dma_start(bf_sb[:], BF.ap().rearrange("(t p) -> p t", p=128))
            make_identity(nc, ident[:])

            env = dict(locals())
            if dma_outside:
                _emit_input_dmas(nc, env)
            loop_cm = (tc.For_i(0, bench_reps, 1) if bench_reps
                       else contextlib.nullcontext())
            with loop_cm:
                _emit_body(nc, tc, mybir, env, skip_dmas=dma_outside)
    nc.finalize()
    return nc


DMA_CHUNKED = True


def _emit_input_dmas(nc, env):
    xq_sb, xc_sb = env["xq_sb"], env["xc_sb"]
    xq_d, xc_d = env["xq_d"], env["xc_d"]
    if not DMA_CHUNKED:
        for ct in range(CT):
            nc.sync.dma_start(xq_sb[:, ct], xq_d[:, ct])
        for ct in range(CT):
            nc.sync.dma_start(xc_sb[:, ct], xc_d[:, ct])
        return
    for qc in range(QH // 512):
        for ct in range(CT):
            s = slice(qc * 512, (qc + 1) * 512)
            nc.sync.dma_start(xq_sb[:, ct, s], xq_d[:, ct, s])
    for kc in range(KC):
        for ct in range(CT):
            s = slice(kc * 512, (kc + 1) * 512)
            nc.sync.dma_start(xc_sb[:, ct, s], xc_d[:, ct, s])


def _emit_body(nc, tc, mybir, env, skip_dmas=False):
    f32 = mybir.dt.float32
    f32r = mybir.dt.float32r
    f16 = mybir.dt.float16
    AF = mybir.ActivationFunctionType
    (xq_sb, xc_sb, wq_sb, wk_sb, wv_sb, wf_sb, bq_sb, bk_sb, bf_sb,
     q_sb, k_sb, vt_sb, ident) = (
        env["xq_sb"], env["xc_sb"], env["wq_sb"], env["wk_sb"], env["wv_sb"],
        env["wf_sb"], env["bq_sb"], env["bk_sb"], env["bf_sb"],
        env["q_sb"], env["k_sb"], env["vt_sb"], env["ident"])
    soft, ptp, outp, mmp, tpp, avp = (
        env["soft"], env["ptp"], env["outp"], env["mmp"], env["tpp"], env["avp"])
    xq_d, xc_d, out_d = env["xq_d"], env["xc_d"], env["out_d"]

    if not skip_dmas:
        _emit_input_dmas(nc, env)

    if True:
            # ---- Q projection: Q[d, n] (scaled by 1/16 via host weights) ----
            for dt in range(CT):
                for qc in range(QH // 512):
                    ps = mmp.tile([128, 512], f32, tag="mm512")
                    for ct in range(CT):
                        nc.tensor.matmul(
                            ps[:], wq_sb[:, ct, dt * 128:(dt + 1) * 128],
                            xq_sb[:, ct, qc * 512:(qc + 1) * 512],
                            start=(ct == 0), stop=(ct == CT - 1))
                    nc.scalar.activation(
                        q_sb[:, dt, qc * 512:(qc + 1) * 512], ps[:],
                        AF.Identity, bias=bq_sb[:, dt:dt + 1])

            # ---- K projection ----
            for dt in range(CT):
                for kc in range(KC):
                    ps = mmp.tile([128, 512], f32, tag="mm512")
                    for ct in range(CT):
                        nc.tensor.matmul(
                            ps[:], wk_sb[:, ct, dt * 128:(dt + 1) * 128],
                            xc_sb[:, ct, kc * 512:(kc + 1) * 512],
                            start=(ct == 0), stop=(ct == CT - 1))
                    nc.scalar.activation(
                        k_sb[:, dt, kc * 512:(kc + 1) * 512], ps[:],
                        AF.Identity, bias=bk_sb[:, dt:dt + 1])

            # ---- V^T: [keys, d] (no bias; folded into conv bias) ----
            for mt in range(NKT):
                ps = mmp.tile([128, 512], f32, tag="mm512")
                for ct in range(CT):
                    nc.tensor.matmul(
                        ps[:, :C], xc_sb[:, ct, mt * 128:(mt + 1) * 128],
                        wv_sb[:, ct],
                        start=(ct == 0), stop=(ct == CT - 1))
                nc.scalar.activation(vt_sb[:, mt], ps[:, :C], AF.Copy)

            # ---- attention + fused conv, per 512-query superblock ----
            for sb in range(NSB):
                pt_sb = ptp.tile([128, NKT, 512], f16, tag="pt")
                for qj in range(4):
                    qb = 4 * sb + qj
                    p_sb = soft.tile([128, N], f16, tag="p")
                    sums = soft.tile([128, KC], f32, tag="sums")
                    # S = Q_blk^T K, chunk by 512 keys; exp + row-sum
                    for kc in range(KC):
                        ps = mmp.tile([128, 512], f32, tag="mm512")
                        for ct in range(CT):
                            nc.tensor.matmul(
                                ps[:], q_sb[:, ct, qb * 128:(qb + 1) * 128],
                                k_sb[:, ct, kc * 512:(kc + 1) * 512],
                                start=(ct == 0), stop=(ct == CT - 1))
                        nc.scalar.activation(
                            p_sb[:, kc * 512:(kc + 1) * 512], ps[:],
                            AF.Exp, accum_out=sums[:, kc:kc + 1])
                    ssum = soft.tile([128, 1], f32, tag="ssum")
                    nc.vector.reduce_sum(ssum[:], sums[:],
                                         axis=mybir.AxisListType.X)
                    rinv = soft.tile([128, 1], f32, tag="rinv")
                    nc.vector.reciprocal(rinv[:], ssum[:])
                    r256 = soft.tile([128, 1], f32, tag="r256")
                    nc.vector.tensor_scalar_mul(r256[:], rinv[:], 256.0)
                    sid = soft.tile([128, 128], f16, tag="sid")
                    nc.vector.tensor_scalar_mul(sid[:], ident[:], r256[:])
                    # PT[k, q] = P[q, k] * 256/rowsum[q] via fp16 matmul
                    for g in range(NKT // 4):
                        tps = tpp.tile([128, 4, 128], f32, tag="tp")
                        for j in range(4):
                            kt = 4 * g + j
                            nc.tensor.matmul(
                                tps[:, j], p_sb[:, kt * 128:(kt + 1) * 128],
                                sid[:], start=True, stop=True)
                        nc.vector.tensor_copy(
                            pt_sb[:, 4 * g:4 * (g + 1),
                                  qj * 128:(qj + 1) * 128], tps[:])

                # A' = sum_k VT[k, :]^T PT[k, :]  -> [256 d, 512 q]
                aps = avp.tile([128, CT, 512], f32, tag="av")
                for kt in range(NKT):
                    for dt in range(CT):
                        nc.tensor.matmul(
                            aps[:, dt], vt_sb[:, kt, dt * 128:(dt + 1) * 128],
                            pt_sb[:, kt],
                            start=(kt == 0), stop=(kt == NKT - 1))
                a_sb = outp.tile([128, CT, 512], f32r, tag="a")
                nc.scalar.activation(a_sb[:], aps[:], AF.Copy)

                # fused conv: out = Wf1 xq + Wf2' A' + bf2
                o_sb = outp.tile([128, CT, 512], f32, tag="o")
                for dt in range(CT):
                    ops = mmp.tile([128, 512], f32, tag="mm512")
                    for kt in range(2 * CT):
                        rhs = (xq_sb[:, kt, sb * 512:(sb + 1) * 512] if kt < CT
                               else a_sb[:, kt - CT])
                        nc.tensor.matmul(
                            ops[:], wf_sb[:, kt, dt * 128:(dt + 1) * 128],
                            rhs, start=(kt == 0), stop=(kt == 2 * CT - 1))
                    nc.scalar.activation(o_sb[:, dt], ops[:],
                                         AF.Identity, bias=bf_sb[:, dt:dt + 1])
                nc.sync.dma_start(out_d[:, :, sb * 512:(sb + 1) * 512], o_sb[:])


def _get_nc(bench_reps=None, dma_outside=False):
    key = ("nc", bench_reps, dma_outside)
    if key not in _CACHE:
        _CACHE[key] = _build(bench_reps, dma_outside)
    return _CACHE[key]


def _in_maps(transformer_features, cnn_features, Wq, bq, Wk, bk, Wv, bv, Wf, bf):
    xt = np.ascontiguousarray(np.asarray(transformer_features, np.float32)
                              .reshape(B, C, N))
    xc = np.ascontiguousarray(np.asarray(cnn_features, np.float32)
                              .reshape(B, C, N))
    Wq = np.asarray(Wq, np.float32)
    Wk = np.asarray(Wk, np.float32)
    Wv = np.asarray(Wv, np.float32)
    Wf = np.asarray(Wf, np.float32)
    bq = np.asarray(bq, np.float32)
    bk = np.asarray(bk, np.float32)
    bv = np.asarray(bv, np.float32)
    bf = np.asarray(bf, np.float32)

    wqt = np.ascontiguousarray(Wq.T / 16.0)
    wkt = np.ascontiguousarray(Wk.T)
    wvt = np.ascontiguousarray(Wv.T)
    wft = np.ascontiguousarray(Wf.T).copy()
    wft[C:] /= 256.0
    bq_s = bq / 16.0
    bf2 = bf + Wf[:, C:] @ bv

    maps = []
    for c in range(NCORES):
        b, h = divmod(c, 2)
        maps.append(dict(
            xq=np.ascontiguousarray(xt[b][:, h * QH:(h + 1) * QH]),
            xc=xc[b],
            wqt=wqt, wkt=wkt, wvt=wvt, wft=wft,
            bq=bq_s, bk=bk, bf=bf2,
        ))
    return maps


def _run(inputs, trace=False):
    from concourse.bass_utils import run_bass_kernel_spmd
    nc = _get_nc()
    maps = _in_maps(**inputs)
    return run_bass_kernel_spmd(nc, maps, list(range(NCORES)), trace=trace)


def kernel(**inputs) -> np.ndarray:
    res = _run(inputs).results
    out = np.empty((B, C, N), np.float32)
    for c in range(NCORES):
        b, h = divmod(c, 2)
        out[b][:, h * QH:(h + 1) * QH] = res[c]["out"]
    return out.reshape(B, C, H, W)



# revision 2
# speedup vs baseline: 1.2961x; 1.2961x over previous
"""CrossAttentionFusion Trainium2 kernel (nn_CrossAttentionFusion__45561013076033).

Full inputs -> full output. Sharding: 8 cores, core c handles batch b=c//2,
query-half h=c%2 (2048 of 4096 queries). Each core holds the full [256,4096]
cnn feature map of its batch (keys/values), its query-half of the transformer
features, and replicated weights.

v2 dataflow (all matmul operands fp16; S computed transposed so exp writes
P^T directly, eliminating transpose matmuls and PSUM->SBUF casts):
  Q = (Wq X_trf + bq)/16        [256, 2048]   q_sb fp16
  K = Wk X_cnn + bk             [256, 4096]   k_sb fp16
  V^T = X_cnn^T Wv^T            [4096, 256]   vt_sb fp16 (bv folded into bf2)
  per 512-query superblock sb, per 128-key tile kt:
    S^T tile = K_kt^T Q_sb      [128, 512]    (lhsT=K tile, rhs=Q block)
    PT[:,kt] = exp(S^T)         fp16, straight from PSUM via ACT
  rowsum: 5-level DVE add tree over the 32 PT tiles -> acc [128, 512],
    then ones-matmul -> PSUM [128,512] colsums replicated on partitions,
    DVE reciprocal -> rinv [128, 512] f32
  A' = sum_kt V^T_kt^T PT_kt    [256, 512]    unnormalized attended, PSUM
  a  = A' * rinv                fp16 (free-axis normalize, replicated rinv)
  out = Wf1 X_trf + Wf2 a + (bf + Wf2 bv)     [256, 512] f32 -> DRAM
"""

import numpy as np

B, C, H, W = 4, 256, 64, 64
N = H * W            # 4096 tokens
NCORES = 8
QH = N // 2          # 2048 queries per core
CT = C // 128        # 2 channel tiles
KC = N // 512        # 8 key chunks of 512
NSB = QH // 512      # 4 query superblocks per core
NKT = N // 128       # 32 key tiles

_CACHE = {}


def _build():
    import concourse.bass as bass
    import concourse.mybir as mybir
    import concourse.tile as tile
    from concourse import bacc

    f32 = mybir.dt.float32
    f16 = mybir.dt.float16
    AF = mybir.ActivationFunctionType

    nc = bacc.Bacc("TRN2", target_bir_lowering=False, debug=True)

    XQ = nc.dram_tensor("xq", [C, QH], f16, kind="ExternalInput")
    XC = nc.dram_tensor("xc", [C, N], f16, kind="ExternalInput")
    WQT = nc.dram_tensor("wqt", [C, C], f16, kind="ExternalInput")
    WKT = nc.dram_tensor("wkt", [C, C], f16, kind="ExternalInput")
    WVT = nc.dram_tensor("wvt", [C, C], f16, kind="ExternalInput")
    WFT = nc.dram_tensor("wft", [2 * C, C], f16, kind="ExternalInput")
    BQ = nc.dram_tensor("bq", [C], f32, kind="ExternalInput")
    BK = nc.dram_tensor("bk", [C], f32, kind="ExternalInput")
    BF = nc.dram_tensor("bf", [C], f32, kind="ExternalInput")
    OUT = nc.dram_tensor("out", [C, QH], f32, kind="ExternalOutput")

    xq_d = XQ.ap().rearrange("(t p) n -> p t n", p=128)
    xc_d = XC.ap().rearrange("(t p) n -> p t n", p=128)
    wq_d = WQT.ap().rearrange("(t p) d -> p t d", p=128)
    wk_d = WKT.ap().rearrange("(t p) d -> p t d", p=128)
    wv_d = WVT.ap().rearrange("(t p) d -> p t d", p=128)
    wf_d = WFT.ap().rearrange("(t p) d -> p t d", p=128)
    out_d = OUT.ap().rearrange("(t p) n -> p t n", p=128)

    with tile.TileContext(nc) as tc:
        with tc.tile_pool(name="persist", bufs=1) as per, \
             tc.tile_pool(name="ptp", bufs=2) as ptp, \
             tc.tile_pool(name="tree", bufs=1) as trp, \
             tc.tile_pool(name="norm", bufs=2) as nrp, \
             tc.tile_pool(name="outp", bufs=2) as outp, \
             tc.tile_pool(name="mm", bufs=4, space="PSUM") as mmp, \
             tc.tile_pool(name="av", bufs=2, space="PSUM") as avp:

            # ---- persistent tiles ----
            xq_sb = per.tile([128, CT, QH], f16)
            xc_sb = per.tile([128, CT, N], f16)
            wq_sb = per.tile([128, CT, C], f16)
            wk_sb = per.tile([128, CT, C], f16)
            wv_sb = per.tile([128, CT, C], f16)
            wf_sb = per.tile([128, 2 * CT, C], f16)
            bq_sb = per.tile([128, CT], f32)
            bk_sb = per.tile([128, CT], f32)
            bf_sb = per.tile([128, CT], f32)
            q_sb = per.tile([128, CT, QH], f16)
            k_sb = per.tile([128, CT, N], f16)
            vt_sb = per.tile([128, NKT, C], f16)
            ones_sb = per.tile([128, 128], f16)

            nc.sync.dma_start(wq_sb[:], wq_d)
            nc.sync.dma_start(wk_sb[:], wk_d)
            nc.sync.dma_start(wv_sb[:], wv_d)
            nc.sync.dma_start(wf_sb[:], wf_d)
            nc.sync.dma_start(bq_sb[:], BQ.ap().rearrange("(t p) -> p t", p=128))
            nc.sync.dma_start(bk_sb[:], BK.ap().rearrange("(t p) -> p t", p=128))
            nc.sync.dma_start(bf_sb[:], BF.ap().rearrange("(t p) -> p t", p=128))
            nc.vector.memset(ones_sb[:], 1.0)

            # ---- input DMAs, chunked 512 columns at a time ----
            for qc in range(QH // 512):
                for ct in range(CT):
                    s = slice(qc * 512, (qc + 1) * 512)
                    nc.sync.dma_start(xq_sb[:, ct, s], xq_d[:, ct, s])
            for kc in range(KC):
                for ct in range(CT):
                    s = slice(kc * 512, (kc + 1) * 512)
                    nc.sync.dma_start(xc_sb[:, ct, s], xc_d[:, ct, s])

            # ---- Q projection: Q[d, n], bias added on DVE ----
            for dt in range(CT):
                for qc in range(QH // 512):
                    s = slice(qc * 512, (qc + 1) * 512)
                    ps = mmp.tile([128, 512], f32, tag="mm512")
                    for ct in range(CT):
                        nc.tensor.matmul(
                            ps[:], wq_sb[:, ct, dt * 128:(dt + 1) * 128],
                            xq_sb[:, ct, s],
                            start=(ct == 0), stop=(ct == CT - 1))
                    nc.vector.tensor_scalar_add(
                        q_sb[:, dt, s], ps[:], bq_sb[:, dt:dt + 1])

            # ---- K projection ----
            for dt in range(CT):
                for kc in range(KC):
                    s = slice(kc * 512, (kc + 1) * 512)
                    ps = mmp.tile([128, 512], f32, tag="mm512")
                    for ct in range(CT):
                        nc.tensor.matmul(
                            ps[:], wk_sb[:, ct, dt * 128:(dt + 1) * 128],
                            xc_sb[:, ct, s],
                            start=(ct == 0), stop=(ct == CT - 1))
                    nc.vector.tensor_scalar_add(
                        k_sb[:, dt, s], ps[:], bk_sb[:, dt:dt + 1])

            # ---- V^T: [keys, d] (no bias; folded into conv bias) ----
            for kt in range(NKT):
                ps = mmp.tile([128, 512], f32, tag="mm512")
                for ct in range(CT):
                    nc.tensor.matmul(
                        ps[:, :C], xc_sb[:, ct, kt * 128:(kt + 1) * 128],
                        wv_sb[:, ct],
                        start=(ct == 0), stop=(ct == CT - 1))
                nc.vector.tensor_copy(vt_sb[:, kt], ps[:, :C])

            # ---- attention + fused conv, per 512-query superblock ----
            for sb in range(NSB):
                qs = slice(sb * 512, (sb + 1) * 512)
                pt = ptp.tile([128, NKT, 512], f16, tag="pt")
                # S^T tiles: lhsT = K tile [d, k], rhs = Q block [d, q]
                for kt in range(NKT):
                    ps = mmp.tile([128, 512], f32, tag="mm512")
                    for ct in range(CT):
                        nc.tensor.matmul(
                            ps[:], k_sb[:, ct, kt * 128:(kt + 1) * 128],
                            q_sb[:, ct, qs],
                            start=(ct == 0), stop=(ct == CT - 1))
                    nc.scalar.activation(pt[:, kt], ps[:], AF.Exp)

                # rowsum: 5-level DVE add tree over kt, then ones-matmul
                s16 = trp.tile([128, 16, 512], f16, tag="s16")
                s8 = trp.tile([128, 8, 512], f16, tag="s8")
                s4 = trp.tile([128, 4, 512], f16, tag="s4")
                s2 = trp.tile([128, 2, 512], f16, tag="s2")
                acc = trp.tile([128, 512], f16, tag="acc")
                nc.vector.tensor_add(s16[:], pt[:, 0:16], pt[:, 16:32])
                nc.vector.tensor_add(s8[:], s16[:, 0:8], s16[:, 8:16])
                nc.vector.tensor_add(s4[:], s8[:, 0:4], s8[:, 4:8])
                nc.vector.tensor_add(s2[:], s4[:, 0:2], s4[:, 2:4])
                nc.vector.tensor_add(acc[:], s2[:, 0], s2[:, 1])

                rs = avp.tile([128, 2, 512], f32, tag="av")
                nc.tensor.matmul(rs[:, 0], ones_sb[:], acc[:],
                                 start=True, stop=True)
                rinv = nrp.tile([128, 512], f32, tag="rinv")
                nc.vector.reciprocal(rinv[:], rs[:, 0])

                # A' = sum_kt VT_kt^T PT_kt -> [256 d, 512 q] unnormalized
                aps = avp.tile([128, 2, 512], f32, tag="av")
                for kt in range(NKT):
                    for dt in range(CT):
                        nc.tensor.matmul(
                            aps[:, dt], vt_sb[:, kt, dt * 128:(dt + 1) * 128],
                            pt[:, kt],
                            start=(kt == 0), stop=(kt == NKT - 1))
                a_sb = outp.tile([128, CT, 512], f16, tag="a")
                for dt in range(CT):
                    nc.vector.tensor_mul(a_sb[:, dt], aps[:, dt], rinv[:])

                # fused conv: out = Wf1 xq + Wf2 a + bf2
                o_sb = outp.tile([128, CT, 512], f32, tag="o")
                for dt in range(CT):
                    ops = mmp.tile([128, 512], f32, tag="mm512")
                    for j in range(2 * CT):
                        rhs = xq_sb[:, j, qs] if j < CT else a_sb[:, j - CT]
                        nc.tensor.matmul(
                            ops[:], wf_sb[:, j, dt * 128:(dt + 1) * 128],
                            rhs, start=(j == 0), stop=(j == 2 * CT - 1))
                    nc.vector.tensor_scalar_add(
                        o_sb[:, dt], ops[:], bf_sb[:, dt:dt + 1])
                nc.sync.dma_start(out_d[:, :, qs], o_sb[:])
    nc.finalize()
    return nc


def _get_nc():
    if "nc" not in _CACHE:
        _CACHE["nc"] = _build()
    return _CACHE["nc"]


def _in_maps(transformer_features, cnn_features, Wq, bq, Wk, bk, Wv, bv, Wf, bf):
    xt = np.ascontiguousarray(np.asarray(transformer_features, np.float32)
                              .reshape(B, C, N))
    xc = np.ascontiguousarray(np.asarray(cnn_features, np.float32)
                              .reshape(B, C, N))
    Wq = np.asarray(Wq, np.float32)
    Wk = np.asarray(Wk, np.float32)
    Wv = np.asarray(Wv, np.float32)
    Wf = np.asarray(Wf, np.float32)
    bq = np.asarray(bq, np.float32)
    bk = np.asarray(bk, np.float32)
    bv = np.asarray(bv, np.float32)
    bf = np.asarray(bf, np.float32)

    wqt = np.ascontiguousarray(Wq.T / 16.0).astype(np.float16)
    wkt = np.ascontiguousarray(Wk.T).astype(np.float16)
    wvt = np.ascontiguousarray(Wv.T).astype(np.float16)
    wft = np.ascontiguousarray(Wf.T).astype(np.float16)
    bq_s = bq / 16.0
    bf2 = bf + Wf[:, C:] @ bv
    xt16 = xt.astype(np.float16)
    xc16 = xc.astype(np.float16)

    maps = []
    for c in range(NCORES):
        b, h = divmod(c, 2)
        maps.append(dict(
            xq=np.ascontiguousarray(xt16[b][:, h * QH:(h + 1) * QH]),
            xc=xc16[b],
            wqt=wqt, wkt=wkt, wvt=wvt, wft=wft,
            bq=bq_s, bk=bk, bf=bf2,
        ))
    return maps


def _run(inputs, trace=False):
    from concourse.bass_utils import run_bass_kernel_spmd
    nc = _get_nc()
    maps = _in_maps(**inputs)
    return run_bass_kernel_spmd(nc, maps, list(range(NCORES)), trace=trace)


def kernel(**inputs) -> np.ndarray:
    res = _run(inputs).results
    out = np.empty((B, C, N), np.float32)
    for c in range(NCORES):
        b, h = divmod(c, 2)
        out[b][:, h * QH:(h + 1) * QH] = res[c]["out"]
    return out.reshape(B, C, H, W)


# revision 8
# speedup vs baseline: 1.5706x; 1.2118x over previous
"""CrossAttentionFusion Trainium2 kernel (nn_CrossAttentionFusion__45561013076033).

Full inputs -> full output. Sharding: 8 cores, core c handles batch b=c//2,
query-half h=c%2 (2048 of 4096 queries). Each core holds the full [256,4096]
cnn feature map of its batch (keys/values), its query-half of the transformer
features, and replicated weights.

v2 dataflow (all matmul operands fp16; S computed transposed so exp writes
P^T directly, eliminating transpose matmuls and PSUM->SBUF casts):
  Q = (Wq X_trf + bq)/16        [256, 2048]   q_sb fp16
  K = Wk X_cnn + bk             [256, 4096]   k_sb fp16
  V^T = X_cnn^T Wv^T            [4096, 256]   vt_sb fp16 (bv folded into bf2)
  per 512-query superblock sb, per 128-key tile kt:
    S^T tile = K_kt^T Q_sb      [128, 512]    (lhsT=K tile, rhs=Q block)
    PT[:,kt] = exp(S^T)         fp16, straight from PSUM via ACT
  rowsum: 5-level DVE add tree over the 32 PT tiles -> acc [128, 512],
    then ones-matmul -> PSUM [128,512] colsums replicated on partitions,
    DVE reciprocal -> rinv [128, 512] f32
  A' = sum_kt V^T_kt^T PT_kt    [256, 512]    unnormalized attended, PSUM
  a  = A' * rinv                fp16 (free-axis normalize, replicated rinv)
  out = Wf1 X_trf + Wf2 a + (bf + Wf2 bv)     [256, 512] f32 -> DRAM
"""

import numpy as np

B, C, H, W = 4, 256, 64, 64
N = H * W            # 4096 tokens
NCORES = 8
QH = N // 2          # 2048 queries per core
CT = C // 128        # 2 channel tiles
KC = N // 512        # 8 key chunks of 512
NSB = QH // 512      # 4 query superblocks per core
NKT = N // 128       # 32 key tiles

_CACHE = {}


def _build():
    import concourse.bass as bass
    import concourse.mybir as mybir
    import concourse.tile as tile
    from concourse import bacc

    f32 = mybir.dt.float32
    f16 = mybir.dt.float16
    AF = mybir.ActivationFunctionType

    nc = bacc.Bacc("TRN2", target_bir_lowering=False, debug=True)

    WARM = nc.dram_tensor("warm", [128, 64], f32, kind="ExternalOutput")
    XQ = nc.dram_tensor("xq", [C, QH], f16, kind="ExternalInput")
    XC = nc.dram_tensor("xc", [C, N], f16, kind="ExternalInput")
    WQT = nc.dram_tensor("wqt", [C, C], f16, kind="ExternalInput")
    WKT = nc.dram_tensor("wkt", [C, C], f16, kind="ExternalInput")
    WVT = nc.dram_tensor("wvt", [C, C], f16, kind="ExternalInput")
    WFT = nc.dram_tensor("wft", [2 * C, C], f16, kind="ExternalInput")
    BQ = nc.dram_tensor("bq", [C], f32, kind="ExternalInput")
    BK = nc.dram_tensor("bk", [C], f32, kind="ExternalInput")
    BF = nc.dram_tensor("bf", [C], f32, kind="ExternalInput")
    OUT = nc.dram_tensor("out", [C, QH], f32, kind="ExternalOutput")

    xq_d = XQ.ap().rearrange("(t p) n -> p t n", p=128)
    xc_d = XC.ap().rearrange("(t p) n -> p t n", p=128)
    wq_d = WQT.ap().rearrange("(t p) d -> p t d", p=128)
    wk_d = WKT.ap().rearrange("(t p) d -> p t d", p=128)
    wv_d = WVT.ap().rearrange("(t p) d -> p t d", p=128)
    wf_d = WFT.ap().rearrange("(t p) d -> p t d", p=128)
    out_d = OUT.ap().rearrange("(t p) n -> p t n", p=128)

    with tile.TileContext(nc) as tc:
        with tc.tile_pool(name="persist", bufs=1) as per, \
             tc.tile_pool(name="ptp", bufs=2) as ptp, \
             tc.tile_pool(name="tree", bufs=1) as trp, \
             tc.tile_pool(name="norm", bufs=2) as nrp, \
             tc.tile_pool(name="outp", bufs=2) as outp, \
             tc.tile_pool(name="mm", bufs=4, space="PSUM") as mmp, \
             tc.tile_pool(name="av", bufs=2, space="PSUM") as avp:

            # ---- persistent tiles ----
            xq_sb = per.tile([128, CT, QH], f16)
            xc_sb = per.tile([128, CT, N], f16)
            wq_sb = per.tile([128, CT, C], f16)
            wk_sb = per.tile([128, CT, C], f16)
            wv_sb = per.tile([128, CT, C], f16)
            wf_sb = per.tile([128, 2 * CT, C], f16)
            bq_sb = per.tile([128, CT], f32)
            bk_sb = per.tile([128, CT], f32)
            bf_sb = per.tile([128, CT], f32)
            q_sb = per.tile([128, CT, QH], f16)
            k_sb = per.tile([128, CT, N], f16)
            vt_sb = per.tile([128, NKT, C], f16)
            ones_sb = per.tile([128, 128], f16)

            nc.sync.dma_start(wq_sb[:], wq_d)
            nc.sync.dma_start(wk_sb[:], wk_d)
            nc.sync.dma_start(wv_sb[:], wv_d)
            nc.sync.dma_start(wf_sb[:], wf_d)
            nc.sync.dma_start(bq_sb[:], BQ.ap().rearrange("(t p) -> p t", p=128))
            nc.sync.dma_start(bk_sb[:], BK.ap().rearrange("(t p) -> p t", p=128))
            nc.sync.dma_start(bf_sb[:], BF.ap().rearrange("(t p) -> p t", p=128))
            nc.vector.memset(ones_sb[:], 1.0)

            # ---- PE warm-up: keep the PE busy from t~0 so the HAM clock
            # gate opens (1.2 -> 2.4 GHz) before the real matmuls start.
            wps = mmp.tile([128, 512], f32, tag="mm512")
            for _ in range(80):
                nc.tensor.matmul(wps[:, :64], ones_sb[:], ones_sb[:, :64],
                                 start=True, stop=True)
            w_sb = per.tile([128, 64], f32)
            nc.scalar.activation(w_sb[:], wps[:, :64], AF.Copy)
            nc.sync.dma_start(WARM.ap(), w_sb[:])

            # ---- inputs + projections, interleaved per 512-column chunk ----
            # Q projection: Q[d, n], bias added on DVE
            for qc in range(QH // 512):
                s = slice(qc * 512, (qc + 1) * 512)
                for ct in range(CT):
                    nc.sync.dma_start(xq_sb[:, ct, s], xq_d[:, ct, s])
                for dt in range(CT):
                    ps = mmp.tile([128, 512], f32, tag="mm512")
                    for ct in range(CT):
                        nc.tensor.matmul(
                            ps[:], wq_sb[:, ct, dt * 128:(dt + 1) * 128],
                            xq_sb[:, ct, s],
                            start=(ct == 0), stop=(ct == CT - 1))
                    nc.vector.tensor_scalar_add(
                        q_sb[:, dt, s], ps[:], bq_sb[:, dt:dt + 1])

            # K + V^T projections per xc chunk
            for kc in range(KC):
                s = slice(kc * 512, (kc + 1) * 512)
                for ct in range(CT):
                    nc.sync.dma_start(xc_sb[:, ct, s], xc_d[:, ct, s])
                for dt in range(CT):
                    ps = mmp.tile([128, 512], f32, tag="mm512")
                    for ct in range(CT):
                        nc.tensor.matmul(
                            ps[:], wk_sb[:, ct, dt * 128:(dt + 1) * 128],
                            xc_sb[:, ct, s],
                            start=(ct == 0), stop=(ct == CT - 1))
                    nc.vector.tensor_scalar_add(
                        k_sb[:, dt, s], ps[:], bk_sb[:, dt:dt + 1])
                # V^T: [keys, d] (no bias; folded into conv bias)
                for j in range(4):
                    kt = 4 * kc + j
                    ps = mmp.tile([128, 512], f32, tag="mm512")
                    for ct in range(CT):
                        nc.tensor.matmul(
                            ps[:, :C], xc_sb[:, ct, kt * 128:(kt + 1) * 128],
                            wv_sb[:, ct],
                            start=(ct == 0), stop=(ct == CT - 1))
                    nc.scalar.activation(vt_sb[:, kt], ps[:, :C], AF.Copy)

            # ---- attention + fused conv, per 512-query superblock ----
            for sb in range(NSB):
                qs = slice(sb * 512, (sb + 1) * 512)
                pt = ptp.tile([128, NKT, 512], f16, tag="pt")
                # S^T tiles: lhsT = K tile [d, k], rhs = Q block [d, q]
                for kt in range(NKT):
                    ps = mmp.tile([128, 512], f32, tag="mm512")
                    for ct in range(CT):
                        nc.tensor.matmul(
                            ps[:], k_sb[:, ct, kt * 128:(kt + 1) * 128],
                            q_sb[:, ct, qs],
                            start=(ct == 0), stop=(ct == CT - 1))
                    nc.scalar.activation(pt[:, kt], ps[:], AF.Exp)

                # rowsum: 5-level DVE add tree over kt, then ones-matmul
                s16 = trp.tile([128, 16, 512], f16, tag="s16")
                s8 = trp.tile([128, 8, 512], f16, tag="s8")
                s4 = trp.tile([128, 4, 512], f16, tag="s4")
                s2 = trp.tile([128, 2, 512], f16, tag="s2")
                acc = trp.tile([128, 512], f16, tag="acc")
                nc.vector.tensor_add(s16[:], pt[:, 0:16], pt[:, 16:32])
                nc.vector.tensor_add(s8[:], s16[:, 0:8], s16[:, 8:16])
                nc.vector.tensor_add(s4[:], s8[:, 0:4], s8[:, 4:8])
                nc.vector.tensor_add(s2[:], s4[:, 0:2], s4[:, 2:4])
                nc.vector.tensor_add(acc[:], s2[:, 0], s2[:, 1])

                rs = mmp.tile([128, 512], f32, tag="mm512")
                nc.tensor.matmul(rs[:], ones_sb[:], acc[:],
                                 start=True, stop=True)
                rinv = nrp.tile([128, 512], f32, tag="rinv")
                nc.vector.reciprocal_approx_fast(rinv[:], rs[:])

                # A' = sum_kt VT_kt^T PT_kt -> [256 d, 512 q] unnormalized
                aps = avp.tile([128, 2, 512], f32, tag="av")
                for kt in range(NKT):
                    for dt in range(CT):
                        nc.tensor.matmul(
                            aps[:, dt], vt_sb[:, kt, dt * 128:(dt + 1) * 128],
                            pt[:, kt],
                            start=(kt == 0), stop=(kt == NKT - 1))
                a_sb = outp.tile([128, CT, 512], f16, tag="a")
                for dt in range(CT):
                    nc.vector.tensor_mul(a_sb[:, dt], aps[:, dt], rinv[:])

                # fused conv: out = Wf1 xq + Wf2 a + bf2
                o_sb = outp.tile([128, CT, 512], f32, tag="o")
                for dt in range(CT):
                    ops = mmp.tile([128, 512], f32, tag="mm512")
                    for j in range(2 * CT):
                        rhs = xq_sb[:, j, qs] if j < CT else a_sb[:, j - CT]
                        nc.tensor.matmul(
                            ops[:], wf_sb[:, j, dt * 128:(dt + 1) * 128],
                            rhs, start=(j == 0), stop=(j == 2 * CT - 1))
                    nc.vector.tensor_scalar_add(
                        o_sb[:, dt], ops[:], bf_sb[:, dt:dt + 1])
                    nc.sync.dma_start(out_d[:, dt, qs], o_sb[:, dt])
    nc.finalize()
    return nc


def _get_nc():
    if "nc" not in _CACHE:
        _CACHE["nc"] = _build()
    return _CACHE["nc"]


def _in_maps(transformer_features, cnn_features, Wq, bq, Wk, bk, Wv, bv, Wf, bf):
    xt = np.ascontiguousarray(np.asarray(transformer_features, np.float32)
                              .reshape(B, C, N))
    xc = np.ascontiguousarray(np.asarray(cnn_features, np.float32)
                              .reshape(B, C, N))
    Wq = np.asarray(Wq, np.float32)
    Wk = np.asarray(Wk, np.float32)
    Wv = np.asarray(Wv, np.float32)
    Wf = np.asarray(Wf, np.float32)
    bq = np.asarray(bq, np.float32)
    bk = np.asarray(bk, np.float32)
    bv = np.asarray(bv, np.float32)
    bf = np.asarray(bf, np.float32)

    wqt = np.ascontiguousarray(Wq.T / 16.0).astype(np.float16)
    wkt = np.ascontiguousarray(Wk.T).astype(np.float16)
    wvt = np.ascontiguousarray(Wv.T).astype(np.float16)
    wft = np.ascontiguousarray(Wf.T).astype(np.float16)
    bq_s = bq / 16.0
    bf2 = bf + Wf[:, C:] @ bv
    xt16 = xt.astype(np.float16)
    xc16 = xc.astype(np.float16)

    maps = []
    for c in range(NCORES):
        b, h = divmod(c, 2)
        maps.append(dict(
            xq=np.ascontiguousarray(xt16[b][:, h * QH:(h + 1) * QH]),
            xc=xc16[b],
            wqt=wqt, wkt=wkt, wvt=wvt, wft=wft,
            bq=bq_s, bk=bk, bf=bf2,
        ))
    return maps


def _run(inputs, trace=False):
    from concourse.bass_utils import run_bass_kernel_spmd
    nc = _get_nc()
    maps = _in_maps(**inputs)
    return run_bass_kernel_spmd(nc, maps, list(range(NCORES)), trace=trace)


def kernel(**inputs) -> np.ndarray:
    res = _run(inputs).results
    out = np.empty((B, C, N), np.float32)
    for c in range(NCORES):
        b, h = divmod(c, 2)
        out[b][:, h * QH:(h + 1) * QH] = res[c]["out"]
    return out.reshape(B, C, H, W)


# revision 13
# speedup vs baseline: 1.7503x; 1.1144x over previous
"""CrossAttentionFusion Trainium2 kernel (nn_CrossAttentionFusion__45561013076033).

Full inputs -> full output. Sharding: 8 cores, core c handles batch b=c//2,
query-half h=c%2 (2048 of 4096 queries). Each core holds the full [256,4096]
cnn feature map of its batch (keys/values), its query-half of the transformer
features, and replicated weights.

v2 dataflow (all matmul operands fp16; S computed transposed so exp writes
P^T directly, eliminating transpose matmuls and PSUM->SBUF casts):
  Q = (Wq X_trf + bq)/16        [256, 2048]   q_sb fp16
  K = Wk X_cnn + bk             [256, 4096]   k_sb fp16
  V^T = X_cnn^T Wv^T            [4096, 256]   vt_sb fp16 (bv folded into bf2)
  per 512-query superblock sb, per 128-key tile kt:
    S^T tile = K_kt^T Q_sb      [128, 512]    (lhsT=K tile, rhs=Q block)
    PT[:,kt] = exp(S^T)         fp16, straight from PSUM via ACT
  rowsum: 5-level DVE add tree over the 32 PT tiles -> acc [128, 512],
    then ones-matmul -> PSUM [128,512] colsums replicated on partitions,
    DVE reciprocal -> rinv [128, 512] f32
  A' = sum_kt V^T_kt^T PT_kt    [256, 512]    unnormalized attended, PSUM
  a  = A' * rinv                fp16 (free-axis normalize, replicated rinv)
  out = Wf1 X_trf + Wf2 a + (bf + Wf2 bv)     [256, 512] f32 -> DRAM
"""

import numpy as np

B, C, H, W = 4, 256, 64, 64
N = H * W            # 4096 tokens
NCORES = 8
QH = N // 2          # 2048 queries per core
CT = C // 128        # 2 channel tiles
KC = N // 512        # 8 key chunks of 512
NSB = QH // 512      # 4 query superblocks per core
NKT = N // 128       # 32 key tiles

_CACHE = {}


def _build():
    import concourse.bass as bass
    import concourse.mybir as mybir
    import concourse.tile as tile
    from concourse import bacc

    f32 = mybir.dt.float32
    f16 = mybir.dt.float16
    f8 = mybir.dt.float8e4
    DR = mybir.MatmulPerfMode.DoubleRow
    AF = mybir.ActivationFunctionType

    nc = bacc.Bacc("TRN2", target_bir_lowering=False, debug=True)

    WARM = nc.dram_tensor("warm", [128, 64], f32, kind="ExternalOutput")
    XQ = nc.dram_tensor("xq", [C, QH], f16, kind="ExternalInput")
    XC = nc.dram_tensor("xc", [C, N], f16, kind="ExternalInput")
    WQT = nc.dram_tensor("wqt", [C, C], f16, kind="ExternalInput")
    WKT = nc.dram_tensor("wkt", [C, C], f16, kind="ExternalInput")
    WVT = nc.dram_tensor("wvt", [C, C], f16, kind="ExternalInput")
    WFT = nc.dram_tensor("wft", [2 * C, C], f16, kind="ExternalInput")
    BQ = nc.dram_tensor("bq", [C], f32, kind="ExternalInput")
    BK = nc.dram_tensor("bk", [C], f32, kind="ExternalInput")
    BF = nc.dram_tensor("bf", [C], f32, kind="ExternalInput")
    OUT = nc.dram_tensor("out", [C, QH], f32, kind="ExternalOutput")

    xq_d = XQ.ap().rearrange("(t p) n -> p t n", p=128)
    xc_d = XC.ap().rearrange("(t p) n -> p t n", p=128)
    wq_d = WQT.ap().rearrange("(t p) d -> p t d", p=128)
    wk_d = WKT.ap().rearrange("(t p) d -> p t d", p=128)
    wv_d = WVT.ap().rearrange("(t p) d -> p t d", p=128)
    wf_d = WFT.ap().rearrange("(t p) d -> p t d", p=128)
    out_d = OUT.ap().rearrange("(t p) n -> p t n", p=128)

    with tile.TileContext(nc) as tc:
        with tc.tile_pool(name="persist", bufs=1) as per, \
             tc.tile_pool(name="ptp", bufs=2) as ptp, \
             tc.tile_pool(name="norm", bufs=2) as nrp, \
             tc.tile_pool(name="outp", bufs=2) as outp, \
             tc.tile_pool(name="mm", bufs=2, space="PSUM") as mmp, \
             tc.tile_pool(name="av", bufs=2, space="PSUM") as avp:

            # ---- persistent tiles ----
            xq_sb = per.tile([128, CT, QH], f16)
            xc_sb = per.tile([128, CT, N], f16)
            wq_sb = per.tile([128, CT, C], f16)
            wk_sb = per.tile([128, CT, C], f16)
            wv_sb = per.tile([128, CT, C], f16)
            wf_sb = per.tile([128, 2 * CT, C], f16)
            bq_sb = per.tile([128, CT], f32)
            bk_sb = per.tile([128, CT], f32)
            bf_sb = per.tile([128, CT], f32)
            q_sb = per.tile([128, CT, QH], f8)
            k_sb = per.tile([128, CT, N], f8)
            vt_sb = per.tile([128, NKT, C], f8)
            ones_sb = per.tile([128, 2, 128], f8)
            onesw_sb = per.tile([128, 128], f16)

            nc.sync.dma_start(wq_sb[:], wq_d)
            nc.sync.dma_start(wk_sb[:], wk_d)
            nc.sync.dma_start(wv_sb[:], wv_d)
            nc.sync.dma_start(wf_sb[:], wf_d)
            nc.sync.dma_start(bq_sb[:], BQ.ap().rearrange("(t p) -> p t", p=128))
            nc.sync.dma_start(bk_sb[:], BK.ap().rearrange("(t p) -> p t", p=128))
            nc.sync.dma_start(bf_sb[:], BF.ap().rearrange("(t p) -> p t", p=128))
            nc.vector.memset(ones_sb[:], 1.0)
            nc.vector.memset(onesw_sb[:], 1.0)

            # ---- PE warm-up: keep the PE busy from t~0 so the HAM clock
            # gate opens (1.2 -> 2.4 GHz) before the real matmuls start.
            wps = mmp.tile([128, 512], f32, tag="mm512")
            for _ in range(80):
                nc.tensor.matmul(wps[:, :64], onesw_sb[:], onesw_sb[:, :64],
                                 start=True, stop=True)
            w_sb = per.tile([128, 64], f32)
            nc.scalar.activation(w_sb[:], wps[:, :64], AF.Copy)
            nc.sync.dma_start(WARM.ap(), w_sb[:])

            # ---- inputs + projections, interleaved per 512-column chunk ----
            # Q projection: Q[d, n], bias added on DVE
            for qc in range(QH // 512):
                s = slice(qc * 512, (qc + 1) * 512)
                for ct in range(CT):
                    nc.sync.dma_start(xq_sb[:, ct, s], xq_d[:, ct, s])
                for dt in range(CT):
                    ps = mmp.tile([128, 512], f32, tag="mm512")
                    for ct in range(CT):
                        nc.tensor.matmul(
                            ps[:], wq_sb[:, ct, dt * 128:(dt + 1) * 128],
                            xq_sb[:, ct, s],
                            start=(ct == 0), stop=(ct == CT - 1))
                    nc.vector.tensor_scalar_add(
                        q_sb[:, dt, s], ps[:], bq_sb[:, dt:dt + 1])

            # K + V^T projections per xc chunk
            for kc in range(KC):
                s = slice(kc * 512, (kc + 1) * 512)
                for ct in range(CT):
                    nc.sync.dma_start(xc_sb[:, ct, s], xc_d[:, ct, s])
                for dt in range(CT):
                    ps = mmp.tile([128, 512], f32, tag="mm512")
                    for ct in range(CT):
                        nc.tensor.matmul(
                            ps[:], wk_sb[:, ct, dt * 128:(dt + 1) * 128],
                            xc_sb[:, ct, s],
                            start=(ct == 0), stop=(ct == CT - 1))
                    nc.vector.tensor_scalar_add(
                        k_sb[:, dt, s], ps[:], bk_sb[:, dt:dt + 1])
                # V^T: [keys, d] (no bias; folded into conv bias)
                for j in range(4):
                    kt = 4 * kc + j
                    ps = mmp.tile([128, 512], f32, tag="mm512")
                    for ct in range(CT):
                        nc.tensor.matmul(
                            ps[:, :C], xc_sb[:, ct, kt * 128:(kt + 1) * 128],
                            wv_sb[:, ct],
                            start=(ct == 0), stop=(ct == CT - 1))
                    nc.scalar.activation(vt_sb[:, kt], ps[:, :C], AF.Copy)

            # ---- attention + fused conv, per 512-query superblock ----
            for sb in range(NSB):
                qs = slice(sb * 512, (sb + 1) * 512)
                pt = ptp.tile([128, NKT, 512], f8, tag="pt")
                # S^T tiles via fp8 DoubleRow: one matmul does the full
                # 256-deep contraction; exp over kt-pairs (1024 wide)
                for g in range(NKT // 2):
                    ps = mmp.tile([128, 1024], f32, tag="mm512")
                    for j in range(2):
                        kt = 2 * g + j
                        nc.tensor.matmul(
                            ps[:, j * 512:(j + 1) * 512],
                            k_sb[:, :, kt * 128:(kt + 1) * 128],
                            q_sb[:, :, qs],
                            start=True, stop=True, perf_mode=DR)
                    nc.scalar.activation(pt[:, 2 * g:2 * (g + 1)], ps[:],
                                         AF.Exp)

                # rowsum via fp8 DoubleRow ones-matmuls, then fast reciprocal
                rs = mmp.tile([128, 1024], f32, tag="mm512")
                for g in range(NKT // 2):
                    nc.tensor.matmul(
                        rs[:, :512], ones_sb[:],
                        pt[:, 2 * g:2 * (g + 1)],
                        start=(g == 0), stop=(g == NKT // 2 - 1),
                        perf_mode=DR)
                rinv = nrp.tile([128, 512], f32, tag="rinv")
                nc.vector.reciprocal_approx_fast(rinv[:], rs[:, :512])

                # A' = sum_kt VT_kt^T PT_kt -> [256 d, 512 q] unnormalized
                aps = avp.tile([128, 2, 512], f32, tag="av")
                for g in range(NKT // 2):
                    for dt in range(CT):
                        nc.tensor.matmul(
                            aps[:, dt],
                            vt_sb[:, 2 * g:2 * (g + 1),
                                  dt * 128:(dt + 1) * 128],
                            pt[:, 2 * g:2 * (g + 1)],
                            start=(g == 0), stop=(g == NKT // 2 - 1),
                            perf_mode=DR)
                a_sb = outp.tile([128, CT, 512], f16, tag="a")
                for dt in range(CT):
                    nc.vector.tensor_mul(a_sb[:, dt], aps[:, dt], rinv[:])

                # fused conv: out = Wf1 xq + Wf2 a + bf2
                o_sb = outp.tile([128, CT, 512], f32, tag="o")
                for dt in range(CT):
                    ops = mmp.tile([128, 512], f32, tag="mm512")
                    for j in range(2 * CT):
                        rhs = xq_sb[:, j, qs] if j < CT else a_sb[:, j - CT]
                        nc.tensor.matmul(
                            ops[:], wf_sb[:, j, dt * 128:(dt + 1) * 128],
                            rhs, start=(j == 0), stop=(j == 2 * CT - 1))
                    nc.vector.tensor_scalar_add(
                        o_sb[:, dt], ops[:], bf_sb[:, dt:dt + 1])
                    nc.sync.dma_start(out_d[:, dt, qs], o_sb[:, dt])
    nc.finalize()
    return nc


def _get_nc():
    if "nc" not in _CACHE:
        _CACHE["nc"] = _build()
    return _CACHE["nc"]


def _in_maps(transformer_features, cnn_features, Wq, bq, Wk, bk, Wv, bv, Wf, bf):
    xt = np.ascontiguousarray(np.asarray(transformer_features, np.float32)
                              .reshape(B, C, N))
    xc = np.ascontiguousarray(np.asarray(cnn_features, np.float32)
                              .reshape(B, C, N))
    Wq = np.asarray(Wq, np.float32)
    Wk = np.asarray(Wk, np.float32)
    Wv = np.asarray(Wv, np.float32)
    Wf = np.asarray(Wf, np.float32)
    bq = np.asarray(bq, np.float32)
    bk = np.asarray(bk, np.float32)
    bv = np.asarray(bv, np.float32)
    bf = np.asarray(bf, np.float32)

    wqt = np.ascontiguousarray(Wq.T / 16.0).astype(np.float16)
    wkt = np.ascontiguousarray(Wk.T).astype(np.float16)
    wvt = np.ascontiguousarray(Wv.T).astype(np.float16)
    wft = np.ascontiguousarray(Wf.T).astype(np.float16)
    bq_s = bq / 16.0
    bf2 = bf + Wf[:, C:] @ bv
    xt16 = xt.astype(np.float16)
    xc16 = xc.astype(np.float16)

    maps = []
    for c in range(NCORES):
        b, h = divmod(c, 2)
        maps.append(dict(
            xq=np.ascontiguousarray(xt16[b][:, h * QH:(h + 1) * QH]),
            xc=xc16[b],
            wqt=wqt, wkt=wkt, wvt=wvt, wft=wft,
            bq=bq_s, bk=bk, bf=bf2,
        ))
    return maps


def _run(inputs, trace=False):
    from concourse.bass_utils import run_bass_kernel_spmd
    nc = _get_nc()
    maps = _in_maps(**inputs)
    return run_bass_kernel_spmd(nc, maps, list(range(NCORES)), trace=trace)


def kernel(**inputs) -> np.ndarray:
    res = _run(inputs).results
    out = np.empty((B, C, N), np.float32)
    for c in range(NCORES):
        b, h = divmod(c, 2)
        out[b][:, h * QH:(h + 1) * QH] = res[c]["out"]
    return out.reshape(B, C, H, W)
